# revision 58
# baseline (speedup 1.0000x reference)
"""NePuEncoder Bass/Tile kernel for 8 Trainium2 NeuronCores.

Sharding: query-parallel. Core c handles batch b=c//4, queries qo=(c%4)*96 ..
qo+96 of that batch. Channel-major layout [128 chan, keys] throughout.

v3 pipeline (software-pipelined 2 pairs ahead, per block):
  PE : hpre = G1P@trig + fp8-DoubleRow inject of EKW=Wg1@EK (precomputed/blk)
       logits = Wg2@h ; pos = Ps@trig + fp8-DoubleRow inject of VK
  ACT: one exp per 2-query logit pair (bias bg2 per-block) with accum_out
       T = S0(2k)+S0(2k+1); even-query relu + cols [0,CA) of odd relu
  DVE: S1 accumulation (stt), S0(2k) via tensor_reduce on the even w half
       (odd S0 recovered as T - S0even at the tail), odd-relu cols [CA,384)
Stage 1: pairwise trig by angle addition: frac(s*xk) key phases once, one
Sin builds [sin;cos](2*pi*pf); per chunk ONE bf16 matmul whose lhsT carries
cos/sin(2*pi*of_q) host-side, evictions alternate ACT/DVE. BN rsqrt uses
Ln+Exp under one activation table (natural_log_exp_and_others) loaded once.
"""
import sys

sys.path.insert(0, "/opt/trn_rl_repo")

import numpy as np
import ml_dtypes

B, N, D, DS, LAT, FD, NF = 2, 384, 128, 3, 256, 1, 2
NB = NF + 1
NQ = 96                  # queries per core
RELU_ACT_EVEN = True     # even queries relu on ACT, odd on DVE (per-pair AD)
FREQS = np.linspace(1.0, 32.0, 5).astype(np.float64)
EPS = 1e-5
TWO_PI = float(2 * np.pi)
C_ROUND = float(3 << 22)  # 2^23 + 2^22: fp32 round-to-nearest-even trick
FP8_INJ = True           # fp8 DoubleRow injection matmuls for EKW/VK
FP8_SCALE = 8.0
S0_SPLIT_CA = 178        # odd-relu columns on ACT; rest on DVE
EXP_PAIR = True          # one exp op per 2 queries

BF = ml_dtypes.bfloat16
F16 = np.float16

# cols24 column map (per-channel [128,1] fp32 constants)
_CC = {"bg2": 0, "gam": 3, "bet": 6, "emb1": 9, "emb2": 11, "emg": 13,
       "embe": 15, "encb": 17, "f1b": 18, "f2b": 20}


def _bf(x):
    return np.ascontiguousarray(np.asarray(x, np.float32).astype(BF))


def _f32(x):
    return np.ascontiguousarray(np.asarray(x, np.float32))


def _f16(x):
    return np.ascontiguousarray(np.asarray(x, np.float32).astype(F16))


def _wpe_split(Wpe):
    """W_s [128,30] trig cols (sin-sign absorbed for npd = xk - xq), W_d [128,3]."""
    Ws = np.zeros((D, 30), np.float32)
    for j in range(3):
        for t in range(10):
            r = 10 * j + t
            if t < 5:
                Ws[:, r] = -Wpe[:, 3 + 6 * t + j]
            else:
                Ws[:, r] = Wpe[:, 3 + 6 * (t - 5) + 3 + j]
    return Ws, Wpe[:, 0:3].astype(np.float32)


_CACHE = {}


def _build(variant="spmd"):
    if variant in _CACHE:
        return _CACHE[variant]

    import concourse.bacc as bacc
    import concourse.bass as bass
    import concourse.tile as tile
    from concourse import mybir

    F32, BF16 = mybir.dt.float32, mybir.dt.bfloat16
    FP16, FP8 = mybir.dt.float16, mybir.dt.float8e4
    AF = mybir.ActivationFunctionType
    OP = mybir.AluOpType
    DR = mybir.MatmulPerfMode.DoubleRow

    nc = bacc.Bacc(None, target_bir_lowering=False,
                   num_devices=(8 if variant == "spmd" else 1))

    def din(name, shape, dt=BF16):
        return nc.dram_tensor(name, shape, dt, kind="ExternalInput")

    # per-core inputs
    xk18 = din("xk18", [18, 384])
    L_OF2 = din("L_OF2", [47, 32 * 96])
    xk_b = din("xk_b", [3, 384])
    xq_b = din("xq_b", [3, 96])
    feats_q = din("feats_q", [1, 96])
    # replicated inputs
    feats_row = din("feats_row", [1, 768])
    SD18 = din("SD18", [18, 47])
    i128h8 = din("i128h8", [128, 256], FP8)
    i128 = din("i128", [128, 128])
    L_G1P3 = din("L_G1P3", [NB, 96, 128])
    L_Ps3 = din("L_Ps3", [NB, 96, 128])
    L_Wg1 = din("L_Wg1", [NB, 128, 128])
    L_Wg2 = din("L_Wg2", [NB, 128, 128])
    L_nWk = din("L_nWk", [NB, 128, 128])
    L_Wv = din("L_Wv", [NB, 128, 128])
    L_G1Q = din("L_G1Q", [NB, 128, 128])
    L_pd9 = din("L_pd9", [3, 9 * 128])     # nPd4(3) | G1Pd4(3) | Pd4(3)
    rows2 = din("rows2", [1, 896])         # c1(3) | bpe(3) | enc, row vectors
    cols24 = din("cols24", [128, 24], F32)
    L_em1 = din("L_em1", [NF, 128, 128])
    L_em2 = din("L_em2", [NF, 128, 128])
    L_f1 = din("L_f1", [2, 128, 128])
    L_f2 = din("L_f2", [2, 2, 128, 128])

    out_d = nc.dram_tensor("out", [2, 256], F32, kind="ExternalOutput")
    RG = [[0, 1, 2, 3, 4, 5, 6, 7]]

    with tile.TileContext(nc) as tc:
        with (
            tc.tile_pool(name="sing", bufs=1) as sing,
            tc.tile_pool(name="fpool", bufs=2) as fpool,
            tc.tile_pool(name="blk", bufs=2) as blk,
            tc.tile_pool(name="hp", bufs=6) as hp,
            tc.tile_pool(name="wp", bufs=2) as wp,
            tc.tile_pool(name="wide", bufs=2) as wide,
            tc.tile_pool(name="smalls", bufs=4) as smalls,
            tc.tile_pool(name="dram", bufs=1, space="DRAM") as dram,
        ):
            _dmaq = [nc.sync, nc.gpsimd]
            _qi = [0]

            def _q():
                e = _dmaq[_qi[0] % len(_dmaq)]
                _qi[0] += 1
                return e

            def load(src, shape, dt=BF16, pool=sing, tag=None):
                t = pool.tile(shape, dt, tag=tag, name=tag or "ld")
                _q().dma_start(out=t, in_=src)
                return t

            def loadfam(srcT, nblk, tag, np_=128):
                t = sing.tile([np_, nblk * 128], BF16, tag=tag, name=tag)
                ap = srcT[:]
                s = bass.AP(tensor=ap.tensor, offset=ap.offset,
                            ap=[[128, np_], [np_ * 128, nblk], [1, 128]])
                _q().dma_start(out=t.rearrange("p (i c) -> p i c", i=nblk), in_=s)
                return [t[:, i * 128:(i + 1) * 128] for i in range(nblk)]

            # stage-1 critical loads first
            xkf_sb = load(xk18[:], [18, 384], tag="xkf")
            sd_sb = load(SD18[:], [18, 47], tag="sd")
            lof_sb = load(L_OF2[:], [47, 32 * 96], tag="lof")

            def load_weights():
                o = {}
                o["xkb"] = load(xk_b[:], [3, 384], tag="sxkb")
                o["xqb"] = load(xq_b[:], [3, 96], tag="sxqb")
                o["featsq"] = load(feats_q[:], [1, 96], tag="sfq")
                o["feats"] = load(feats_row[:], [1, 768], tag="sfr")
                o["i128h8"] = load(i128h8[:], [128, 256], FP8, tag="i128h8")
                if not FP8_INJ:
                    o["i128"] = load(i128[:], [128, 128], tag="si128")
                o["G1P"] = loadfam(L_G1P3, NB, "g1p3", np_=96)
                o["Ps"] = loadfam(L_Ps3, NB, "ps3", np_=96)
                o["Wg1"] = loadfam(L_Wg1, NB, "wg1")
                o["Wg2"] = loadfam(L_Wg2, NB, "wg2")
                o["nWk"] = loadfam(L_nWk, NB, "nwk")
                o["Wv"] = loadfam(L_Wv, NB, "wv")
                o["G1Q"] = loadfam(L_G1Q, NB, "g1q")
                pd9 = load(L_pd9[:], [3, 9 * 128], tag="pd9")
                o["nPd4"] = [pd9[:, 128 * i:128 * (i + 1)] for i in range(3)]
                o["G1Pd4"] = [pd9[:, 128 * (3 + i):128 * (4 + i)] for i in range(3)]
                o["Pd4"] = [pd9[:, 128 * (6 + i):128 * (7 + i)] for i in range(3)]
                r2 = load(rows2[:], [1, 896], tag="rows2")
                o["c1"] = [r2[0:1, 128 * i:128 * (i + 1)] for i in range(3)]
                o["bpe"] = [r2[0:1, 128 * (3 + i):128 * (4 + i)] for i in range(3)]
                o["enc"] = r2[0:1, 768:896]
                c24 = load(cols24[:], [128, 24], F32, tag="cols24")
                cc = lambda k, i=0: c24[:, _CC[k] + i:_CC[k] + i + 1]
                o["bg2"] = [cc("bg2", i) for i in range(3)]
                o["gam"] = [cc("gam", i) for i in range(3)]
                o["bet"] = [cc("bet", i) for i in range(3)]
                o["emb1"] = [cc("emb1", j) for j in range(NF)]
                o["emb2"] = [cc("emb2", j) for j in range(NF)]
                o["emg"] = [cc("emg", j) for j in range(NF)]
                o["embe"] = [cc("embe", j) for j in range(NF)]
                o["encb"] = cc("encb")
                o["f1b"] = [cc("f1b", h) for h in range(2)]
                o["f2b"] = [cc("f2b", h) for h in range(2)]
                o["em1"] = loadfam(L_em1, NF, "em1")
                o["em2"] = loadfam(L_em2, NF, "em2")
                o["f1"] = loadfam(L_f1, 2, "f1")
                f2t = loadfam(L_f2, 4, "f2")
                o["f2"] = [[f2t[2 * h + k] for k in range(2)] for h in range(2)]
                return o

            ones96 = sing.tile([1, 96], BF16, tag="ones96")
            nc.vector.memset(ones96, 1.0)
            eps_t = sing.tile([128, 1], F32, tag="epst")
            nc.vector.memset(eps_t, EPS)
            dumA = sing.tile([128, 1], BF16, tag="dumA")
            dumP = sing.tile([128, 1], BF16, tag="dumP")

            trig_all = sing.tile([96, 32 * 384], BF16, tag="trigall",
                                 name="trig_all")

            # ---------- stage 1: separable pairwise trig -------------------
            # key phases p[(5j+t),k] = s_t * xk[j,k]; pf = p - round(p);
            # sc = [sin(2*pi*pf); cos(2*pi*pf)] once.  Per chunk the
            # per-query offset trig is folded into the lhsT (host), so
            # trig = sin(2*pi*(pf+of)) comes straight out of one bf16
            # matmul via angle addition; evict to bf16 alternates ACT/DVE.
            with (
                tc.tile_pool(name="s1aux", bufs=1) as s1aux,
                tc.tile_pool(name="s1ps", bufs=4, space="PSUM") as s1ps,
            ):
                pA = s1ps.tile([128, 2, 512], F32, tag="r2", name="pA")
                nc.tensor.matmul(pA[0:47, 0, 0:384], sd_sb[:, :], xkf_sb,
                                 start=True, stop=True)
                nP = s1aux.tile([47, 384], F32, tag="nP")
                nc.vector.tensor_scalar(out=nP, in0=pA[0:47, 0, 0:384],
                                        scalar1=C_ROUND, scalar2=C_ROUND,
                                        op0=OP.add, op1=OP.subtract)
                pf_t = s1aux.tile([47, 384], F32, tag="pf")
                nc.vector.scalar_tensor_tensor(
                    out=pf_t, in0=pA[0:47, 0, 0:384], scalar=1.0,
                    in1=nP, op0=OP.mult, op1=OP.subtract)
                b47 = s1aux.tile([47, 1], F32, tag="b47")
                nc.vector.memset(b47, 0.0)
                nc.vector.memset(b47[32:47, :], float(np.pi / 2))
                sc_t = s1aux.tile([47, 384], BF16, tag="sc")
                nc.scalar.activation(out=sc_t, in_=pf_t, func=AF.Sin,
                                     bias=b47, scale=TWO_PI)

                for g in range(16):
                    rt = s1ps.tile([128, 2, 512], F32, tag="r2", name="rt")
                    for u in range(2):
                        c = 2 * g + u
                        nc.tensor.matmul(rt[0:96, u, 0:384],
                                         lof_sb[:, 96 * c:96 * (c + 1)],
                                         sc_t[:, :], start=True, stop=True)
                    ts = trig_all[:, 768 * g:768 * (g + 1)]
                    ts3 = ts.rearrange("p (u k) -> p u k", u=2)
                    if g % 2 == 0:
                        nc.scalar.activation(out=ts3, in_=rt[0:96, :, 0:384],
                                             func=AF.Copy, bias=0.0, scale=1.0)
                    else:
                        nc.vector.tensor_scalar(out=ts3,
                                                in0=rt[0:96, :, 0:384],
                                                scalar1=1.0, scalar2=None,
                                                op0=OP.mult)
            tc.no_sync_barrier()

            # lock the ACT table to the exp+ln set for the rest of the kernel
            try:
                from concourse.hw_specs import get_activation_tables
                _tabs = list(get_activation_tables(nc.m.arch))
                _idx = _tabs.index("natural_log_exp_and_others")
                nc.scalar.add_instruction(mybir.InstLoadActFuncSet(
                    name=nc.scalar.bass.get_next_instruction_name(),
                    act_func_set_id=_idx, ins=[], outs=[]))
            except Exception:
                pass

            W = load_weights()

            def tsl(m):
                r = m % 3
                return trig_all[32 * r:32 * r + 30,
                                384 * (m // 3):384 * (m // 3 + 1)]

            with (
                tc.tile_pool(name="ps_a", bufs=4, space="PSUM") as ps_a,
                tc.tile_pool(name="ps_b", bufs=2, space="PSUM") as ps_b,
                tc.tile_pool(name="ps_g", bufs=1, space="PSUM") as ps_g,
            ):
                # ---------- initial features ----------
                f_full = fpool.tile([128, 768], BF16, tag="ffull")
                for half in range(2):
                    p = ps_a.tile([128, 512], F32, tag="pa")
                    nc.tensor.matmul(p[:, 0:384], W['enc'],
                                     W['feats'][:, half * 384:(half + 1) * 384],
                                     start=True, stop=True)
                    nc.scalar.activation(out=f_full[:, half * 384:(half + 1) * 384],
                                         in_=p[:, 0:384], func=AF.Identity,
                                         bias=W['encb'], scale=1.0)
                fq = fpool.tile([128, 96], BF16, tag="fq")
                p = ps_a.tile([128, 512], F32, tag="pa")
                nc.tensor.matmul(p[:, 0:96], W['enc'], W['featsq'], start=True,
                                 stop=True)
                nc.scalar.activation(out=fq, in_=p[:, 0:96], func=AF.Identity,
                                     bias=W['encb'], scale=1.0)
                pid = nc.scalar.partition_id()
                fb = fpool.tile([128, 384], BF16, tag="fb")
                with tc.If(pid < 4) as cmp:
                    nc.scalar.copy(fb, f_full[:, 0:384])
                with cmp.Else():
                    nc.scalar.copy(fb, f_full[:, 384:768])

                def affine_evict(src_ap, sc, b2, shape, dt=BF16, tag="aff",
                                 pool=None):
                    t = (pool or fpool).tile(shape, dt, tag=tag)
                    nc.scalar.activation(out=t, in_=src_ap, func=AF.Identity,
                                         bias=b2, scale=sc)
                    return t

                # ---------- transformer blocks ----------
                for i in range(NB):
                    # block consts
                    pa = ps_a.tile([128, 512], F32, tag="pa")
                    nc.tensor.matmul(pa[:, 0:384], W['nWk'][i], fb, start=True,
                                     stop=False)
                    nc.tensor.matmul(pa[:, 0:384], W['nPd4'][i], W['xkb'],
                                     start=False, stop=True)
                    EK = blk.tile([128, 384], BF16, tag="EK")
                    nc.scalar.copy(EK, pa[:, 0:384])

                    if FP8_INJ:
                        pa = ps_a.tile([128, 512], F32, tag="pa")
                        nc.tensor.matmul(pa[:, 0:384], W['Wg1'][i], EK,
                                         start=True, stop=True)
                        ekw8 = blk.tile([128, 2, 384], FP8, tag="ekw8")
                        nc.scalar.activation(
                            out=ekw8,
                            in_=pa[:, 0:384].rearrange("p (o k) -> p o k", o=1)
                                .broadcast_to((128, 2, 384)),
                            func=AF.Copy, bias=0.0, scale=FP8_SCALE)

                    pb = ps_b.tile([128, 512], F32, tag="pb")
                    nc.tensor.matmul(pb[:, 0:384], W['Wv'][i], fb, start=True,
                                     stop=False)
                    nc.tensor.matmul(pb[:, 0:384], W['nPd4'][i], W['xkb'],
                                     start=False, stop=True)
                    if FP8_INJ:
                        vk8 = blk.tile([128, 2, 384], FP8, tag="vk8")
                        nc.scalar.activation(
                            out=vk8,
                            in_=pb[:, 0:384].rearrange("p (o k) -> p o k", o=1)
                                .broadcast_to((128, 2, 384)),
                            func=AF.Copy, bias=0.0, scale=FP8_SCALE)
                        VK = None
                    else:
                        VK = blk.tile([128, 384], BF16, tag="VK")
                        nc.scalar.copy(VK, pb[:, 0:384])

                    pa = ps_a.tile([128, 512], F32, tag="pa")
                    nc.tensor.matmul(pa[:, 0:96], W['G1Q'][i], fq, start=True,
                                     stop=False)
                    nc.tensor.matmul(pa[:, 0:96], W['G1Pd4'][i], W['xqb'],
                                     start=False, stop=False)
                    nc.tensor.matmul(pa[:, 0:96], W['c1'][i], ones96,
                                     start=False, stop=True)
                    QB = blk.tile([128, 96], F32, tag="QB")
                    nc.scalar.copy(QB, pa[:, 0:96])

                    pb = ps_b.tile([128, 512], F32, tag="pb")
                    nc.tensor.matmul(pb[:, 0:96], W['Pd4'][i], W['xqb'],
                                     start=True, stop=False)
                    nc.tensor.matmul(pb[:, 0:96], W['bpe'][i], ones96,
                                     start=False, stop=True)
                    QP = blk.tile([128, 96], F32, tag="QP")
                    nc.scalar.copy(QP, pb[:, 0:96])

                    S1 = blk.tile([128, 96], F32, tag="S1")
                    S0e = blk.tile([128, 48], F32, tag="S0e")
                    Tp = blk.tile([128, 48], F32, tag="Tp")
                    S0 = blk.tile([128, 96], F32, tag="S0")
                    R = blk.tile([128, 96], F32, tag="R")
                    RES = blk.tile([128, 96], F32, tag="RES")
                    payload = blk.tile([128, 128], F32, tag="payload")
                    nc.vector.memset(payload[:, 96:128], 0.0)

                    h_t = [None, None, None, None]  # h tiles mod 4

                    def mm_h(m):
                        hpre = ps_a.tile([128, 512], F32, tag="pa", name="hpre")
                        nc.tensor.matmul(hpre[:, 0:384],
                                         W['G1P'][i][32 * (m % 3):32 * (m % 3) + 30, :],
                                         tsl(m), start=True, stop=False)
                        if FP8_INJ:
                            nc.tensor.matmul(
                                hpre[:, 0:384],
                                W['i128h8'][:, :].rearrange("p (b f) -> p b f", b=2),
                                ekw8[:, :, :], start=False, stop=True,
                                perf_mode=DR, skip_group_check=True)
                        else:
                            nc.tensor.matmul(hpre[:, 0:384], W['Wg1'][i], EK,
                                             start=False, stop=True)
                        return hpre

                    CA = S0_SPLIT_CA

                    def relu_h(m, hpre):
                        t = hp.tile([128, 384], BF16, tag="h", name="h_t")
                        if m % 2 == 0:
                            nc.scalar.activation(out=t, in_=hpre[:, 0:384],
                                                 func=AF.Relu,
                                                 bias=QB[:, m:m + 1], scale=1.0)
                        else:
                            nc.scalar.activation(out=t[:, 0:CA],
                                                 in_=hpre[:, 0:CA],
                                                 func=AF.Relu,
                                                 bias=QB[:, m:m + 1], scale=1.0)
                        h_t[m % 4] = t

                    def relu_h_dve(m, hpre):
                        t = h_t[m % 4]
                        nc.vector.tensor_scalar(
                            out=t[:, CA:384], in0=hpre[:, CA:384],
                            scalar1=QB[:, m:m + 1], scalar2=0.0,
                            op0=OP.add, op1=OP.max)

                    def mm_tail(m, w_ap):
                        pos = ps_b.tile([128, 512], F32, tag="pb", name="pos")
                        nc.tensor.matmul(pos[:, 0:384],
                                         W['Ps'][i][32 * (m % 3):32 * (m % 3) + 30, :],
                                         tsl(m), start=True, stop=False)
                        if FP8_INJ:
                            nc.tensor.matmul(
                                pos[:, 0:384],
                                W['i128h8'][:, :].rearrange("p (b f) -> p b f", b=2),
                                vk8[:, :, :], start=False, stop=True,
                                perf_mode=DR, skip_group_check=True)
                        else:
                            nc.tensor.matmul(pos[:, 0:384], W['i128'], VK,
                                             start=False, stop=True)
                        nc.vector.scalar_tensor_tensor(
                            out=dumA.broadcast_to((128, 384)),
                            in0=pos[:, 0:384], scalar=QP[:, m:m + 1],
                            in1=w_ap, op0=OP.add, op1=OP.mult,
                            accum_out=S1[:, m:m + 1])

                    # software pipeline, 2 pairs ahead: hpre matmuls for
                    # pair kp+2 issue at iter kp; relus follow this iter's exp
                    # so the exp->relu->logits->exp cycle never serializes.
                    for m0 in range(4):
                        hpre0 = mm_h(m0)
                        relu_h(m0, hpre0)
                        if m0 % 2 == 1:
                            relu_h_dve(m0, hpre0)
                    st = smalls.tile([128, 2, 6], F32, tag="bnst2")
                    for kp in range(NQ // 2):
                        lg = ps_g.tile([128, 2, 512], F32, tag="lg", name="lg")
                        nc.tensor.matmul(lg[:, 0, 0:384], W['Wg2'][i],
                                         h_t[(2 * kp) % 4], start=True, stop=True)
                        nc.tensor.matmul(lg[:, 1, 0:384], W['Wg2'][i],
                                         h_t[(2 * kp + 1) % 4], start=True,
                                         stop=True)
                        if kp < NQ // 2 - 2:
                            hA = mm_h(2 * kp + 4)
                            hB = mm_h(2 * kp + 5)
                        w2 = wp.tile([128, 2, 384], BF16, tag="w2", name="w2")
                        nc.scalar.activation(out=w2, in_=lg[:, :, 0:384],
                                             func=AF.Exp, bias=W['bg2'][i],
                                             scale=1.0,
                                             accum_out=Tp[:, kp:kp + 1])
                        if kp < NQ // 2 - 2:
                            relu_h(2 * kp + 4, hA)
                            relu_h(2 * kp + 5, hB)
                        mm_tail(2 * kp, w2[:, 0, :])
                        mm_tail(2 * kp + 1, w2[:, 1, :])
                        nc.vector.tensor_reduce(out=S0e[:, kp:kp + 1],
                                                in_=w2[:, 0, :],
                                                axis=mybir.AxisListType.X,
                                                op=OP.add)
                        if kp < NQ // 2 - 2:
                            relu_h_dve(2 * kp + 5, hB)
                        if kp == 28:
                            # first-half tail math (queries 0-47 complete)
                            sv = S0[:, 0:48].rearrange("p (q t) -> p q t", t=2)
                            nc.vector.tensor_copy(sv[:, :, 0], S0e[:, 0:24])
                            nc.vector.tensor_tensor(out=sv[:, :, 1],
                                                    in0=Tp[:, 0:24],
                                                    in1=S0e[:, 0:24],
                                                    op=OP.subtract)
                            nc.vector.reciprocal(out=R[:, 0:48],
                                                 in_=S0[:, 0:48])
                            nc.vector.tensor_tensor(out=RES[:, 0:48],
                                                    in0=S1[:, 0:48],
                                                    in1=R[:, 0:48], op=OP.mult)
                            nc.vector.tensor_tensor(out=payload[:, 0:48],
                                                    in0=RES[:, 0:48],
                                                    in1=fq[:, 0:48], op=OP.add)
                            nc.vector.bn_stats(out=st[:, 0, :],
                                               in_=payload[:, 0:48])

                    # block tail: second-half o = S1/S0 + fq; stats; gather
                    sv2 = S0[:, 48:96].rearrange("p (q t) -> p q t", t=2)
                    nc.vector.tensor_copy(sv2[:, :, 0], S0e[:, 24:48])
                    nc.vector.tensor_tensor(out=sv2[:, :, 1],
                                            in0=Tp[:, 24:48],
                                            in1=S0e[:, 24:48], op=OP.subtract)
                    nc.vector.reciprocal(out=R[:, 48:96], in_=S0[:, 48:96])
                    nc.vector.tensor_tensor(out=RES[:, 48:96],
                                            in0=S1[:, 48:96], in1=R[:, 48:96],
                                            op=OP.mult)
                    nc.vector.tensor_tensor(out=payload[:, 48:96],
                                            in0=RES[:, 48:96], in1=fq[:, 48:96],
                                            op=OP.add)
                    nc.vector.bn_stats(out=st[:, 1, :], in_=payload[:, 48:96])
                    mv = smalls.tile([128, 2], F32, tag="bnmv")
                    nc.vector.bn_aggr(out=mv, in_=st)
                    nc.vector.tensor_copy(payload[:, 96:97], mv[:, 0:1])
                    msq = smalls.tile([128, 1], F32, tag="msq")
                    nc.vector.tensor_tensor(out=msq, in0=mv[:, 0:1],
                                            in1=mv[:, 0:1], op=OP.mult)
                    nc.vector.tensor_tensor(out=payload[:, 97:98],
                                            in0=mv[:, 1:2], in1=msq, op=OP.add)

                    ag_in = dram.tile([128, 128], F32, tag=f"agin{i}")
                    if variant == "spmd":
                        ag_out = dram.tile([8, 128, 128], F32,
                                           addr_space="Shared", tag=f"agout{i}")
                    else:
                        ag_out = dram.tile([8, 128, 128], F32, tag=f"agout{i}")
                    ago_ap = ag_out[:]
                    if variant == "spmd":
                        nc.gpsimd.dma_start(out=ag_in, in_=payload)
                        nc.gpsimd.collective_compute(
                            "AllGather", OP.bypass, replica_groups=RG,
                            ins=[ag_in[:].opt()], outs=[ag_out[:].opt()])
                    else:
                        bsrc = payload[:, :].rearrange("p (o k) -> p o k", o=1)                             .broadcast_to((128, 8, 128))
                        bdst = bass.AP(tensor=ago_ap.tensor, offset=ago_ap.offset,
                                       ap=[[128, 128], [128 * 128, 8], [1, 128]])
                        nc.sync.dma_start(out=bdst, in_=bsrc)

                    gath = wide.tile([128, 8, 128], F32, tag="gath")
                    src = bass.AP(tensor=ago_ap.tensor, offset=ago_ap.offset,
                                  ap=[[128, 128], [128 * 128, 8], [1, 128]])
                    nc.sync.dma_start(out=gath, in_=src)


                    mg = smalls.tile([128, 1], F32, tag="mg")
                    nc.vector.tensor_reduce(out=mg, in_=gath[:, :, 96],
                                            axis=mybir.AxisListType.X, op=OP.add)
                    nc.vector.tensor_scalar(out=mg, in0=mg, scalar1=0.125,
                                            scalar2=None, op0=OP.mult)
                    e2g = smalls.tile([128, 1], F32, tag="e2g")
                    nc.vector.tensor_reduce(out=e2g, in_=gath[:, :, 97],
                                            axis=mybir.AxisListType.X, op=OP.add)
                    nc.vector.tensor_scalar(out=e2g, in0=e2g, scalar1=0.125,
                                            scalar2=None, op0=OP.mult)
                    var = smalls.tile([128, 1], F32, tag="var")
                    nc.vector.tensor_tensor(out=var, in0=mg, in1=mg, op=OP.mult)
                    nc.vector.tensor_tensor(out=var, in0=e2g, in1=var,
                                            op=OP.subtract)
                    lnv = smalls.tile([128, 1], F32, tag="lnv")
                    nc.scalar.activation(out=lnv, in_=var, func=AF.Ln,
                                         bias=eps_t, scale=1.0)
                    rs = smalls.tile([128, 1], F32, tag="rs")
                    nc.scalar.activation(out=rs, in_=lnv, func=AF.Exp, bias=0.0,
                                         scale=-0.5)
                    sc = smalls.tile([128, 1], F32, tag="sc")
                    nc.vector.tensor_tensor(out=sc, in0=W['gam'][i], in1=rs,
                                            op=OP.mult)
                    b2 = smalls.tile([128, 1], F32, tag="b2")
                    nc.vector.tensor_scalar(out=b2, in0=mg, scalar1=sc,
                                            scalar2=None, op0=OP.mult)
                    nc.vector.tensor_tensor(out=b2, in0=W['bet'][i], in1=b2,
                                            op=OP.subtract)

                    if i > 0:
                        f_full = fpool.tile([128, 768], BF16, tag="ffull")
                        nc.scalar.activation(
                            out=f_full[:, 0:384].rearrange(
                                "p (c k) -> p c k", c=4),
                            in_=gath[:, 0:4, 0:96], func=AF.Identity, bias=b2,
                            scale=sc)
                        nc.vector.scalar_tensor_tensor(
                            out=f_full[:, 384:768].rearrange(
                                "p (c k) -> p c k", c=4),
                            in0=gath[:, 4:8, 0:96], scalar=sc,
                            in1=b2.broadcast_to((128, 4, 96)),
                            op0=OP.mult, op1=OP.add)
                    if i < NB - 1:
                        fq = fpool.tile([128, 96], BF16, tag="fq")
                        nc.vector.tensor_scalar(out=fq, in0=payload[:, 0:96],
                                                scalar1=sc, scalar2=b2,
                                                op0=OP.mult, op1=OP.add)
                    if i == 0:
                        fb = fpool.tile([128, 384], BF16, tag="fb")
                        with tc.If(pid < 4) as cmp:
                            nc.scalar.activation(
                                out=fb.rearrange("p (c k) -> p c k", c=4),
                                in_=gath[:, 0:4, 0:96], func=AF.Identity,
                                bias=b2, scale=sc)
                        with cmp.Else():
                            nc.scalar.activation(
                                out=fb.rearrange("p (c k) -> p c k", c=4),
                                in_=gath[:, 4:8, 0:96], func=AF.Identity,
                                bias=b2, scale=sc)

                    # ---------- MLP ----------
                    if i > 0:
                        j = i - 1

                        def mlp_layer(lw, bias_ap, xin, width, tag):
                            t = wide.tile([128, width], BF16, tag=tag)
                            for h0 in range(0, width, 384):
                                wdt = min(384, width - h0)
                                pp = ps_a.tile([128, 512], F32, tag="pa")
                                nc.tensor.matmul(pp[:, 0:wdt], lw,
                                                 xin[:, h0:h0 + wdt],
                                                 start=True, stop=True)
                                if h0 == 0:
                                    nc.scalar.activation(out=t[:, h0:h0 + wdt],
                                                         in_=pp[:, 0:wdt],
                                                         func=AF.Relu,
                                                         bias=bias_ap,
                                                         scale=1.0)
                                else:
                                    nc.vector.tensor_scalar(
                                        out=t[:, h0:h0 + wdt],
                                        in0=pp[:, 0:wdt], scalar1=bias_ap,
                                        scalar2=0.0, op0=OP.add, op1=OP.max)
                            return t

                        y1f = mlp_layer(W['em1'][j], W['emb1'][j], f_full, 768,
                                        "y1f")
                        y2f = mlp_layer(W['em2'][j], W['emb2'][j], y1f, 768,
                                        "y2f")
                        o2f = wide.tile([128, 768], F32, tag="o2f")
                        nc.vector.tensor_tensor(out=o2f, in0=f_full, in1=y2f,
                                                op=OP.add)
                        if i < NB - 1:
                            y1q = mlp_layer(W['em1'][j], W['emb1'][j], fq, 96,
                                            "y1q")
                            y2q = mlp_layer(W['em2'][j], W['emb2'][j], y1q, 96,
                                            "y2q")
                            o2q = wide.tile([128, 96], F32, tag="o2q")
                            nc.vector.tensor_tensor(out=o2q, in0=fq, in1=y2q,
                                                    op=OP.add)

                        st2 = smalls.tile([128, 2, 6], F32, tag="st2")
                        nc.vector.bn_stats(out=st2[:, 0, :], in_=o2f[:, 0:384])
                        nc.vector.bn_stats(out=st2[:, 1, :], in_=o2f[:, 384:768])
                        mv2 = smalls.tile([128, 2], F32, tag="mv2")
                        nc.vector.bn_aggr(out=mv2, in_=st2)
                        lnv2 = smalls.tile([128, 1], F32, tag="lnv")
                        nc.scalar.activation(out=lnv2, in_=mv2[:, 1:2],
                                             func=AF.Ln, bias=eps_t, scale=1.0)
                        rs2 = smalls.tile([128, 1], F32, tag="rs")
                        nc.scalar.activation(out=rs2, in_=lnv2, func=AF.Exp,
                                             bias=0.0, scale=-0.5)
                        sc2 = smalls.tile([128, 1], F32, tag="sc")
                        nc.vector.tensor_tensor(out=sc2, in0=W['emg'][j],
                                                in1=rs2, op=OP.mult)
                        b22 = smalls.tile([128, 1], F32, tag="b2")
                        nc.vector.tensor_scalar(out=b22, in0=mv2[:, 0:1],
                                                scalar1=sc2, scalar2=None,
                                                op0=OP.mult)
                        nc.vector.tensor_tensor(out=b22, in0=W['embe'][j],
                                                in1=b22, op=OP.subtract)
                        if i == NB - 1:
                            f_full = affine_evict(o2f[:], sc2, b22, [128, 768],
                                                  tag="ffull")
                        if i < NB - 1:
                            fb = fpool.tile([128, 384], BF16, tag="fb")
                            with tc.If(pid < 4) as cmp:
                                nc.scalar.activation(out=fb, in_=o2f[:, 0:384],
                                                     func=AF.Identity,
                                                     bias=b22, scale=sc2)
                            with cmp.Else():
                                nc.scalar.activation(out=fb,
                                                     in_=o2f[:, 384:768],
                                                     func=AF.Identity,
                                                     bias=b22, scale=sc2)
                            fq = fpool.tile([128, 96], BF16, tag="fq")
                            nc.vector.tensor_scalar(out=fq, in0=o2q,
                                                    scalar1=sc2, scalar2=b22,
                                                    op0=OP.mult, op1=OP.add)

                # ---------- final FC + max ----------
                ot = smalls.tile([128, 4], F32, tag="ot")
                for bb in range(2):
                    fbb = f_full[:, bb * 384:(bb + 1) * 384]
                    e1 = []
                    for h in range(2):
                        pp = ps_a.tile([128, 512], F32, tag="pa")
                        nc.tensor.matmul(pp[:, 0:384], W['f1'][h], fbb,
                                         start=True, stop=True)
                        e1t = wide.tile([128, 384], BF16, tag=f"e1{h}")
                        nc.scalar.activation(out=e1t, in_=pp[:, 0:384],
                                             func=AF.Relu, bias=W['f1b'][h],
                                             scale=1.0)
                        e1.append(e1t)
                    for h in range(2):
                        pp = ps_b.tile([128, 512], F32, tag="pb")
                        nc.tensor.matmul(pp[:, 0:384], W['f2'][h][0], e1[0],
                                         start=True, stop=False)
                        nc.tensor.matmul(pp[:, 0:384], W['f2'][h][1], e1[1],
                                         start=False, stop=True)
                        mx = smalls.tile([128, 1], F32, tag="mx")
                        nc.vector.tensor_reduce(out=mx, in_=pp[:, 0:384],
                                                axis=mybir.AxisListType.X,
                                                op=OP.max)
                        nc.vector.tensor_scalar(out=ot[:, 2 * bb + h:2 * bb + h + 1],
                                                in0=mx, scalar1=W['f2b'][h],
                                                scalar2=None, op0=OP.add)
                od_ap = out_d[:]
                dst = bass.AP(tensor=od_ap.tensor, offset=od_ap.offset,
                              ap=[[1, 128], [256, 2], [128, 2]])
                nc.sync.dma_start(out=dst, in_=ot)

    nc.compile()
    _CACHE[variant] = nc
    return nc


def _split3(a):
    """3-level bf16 split (hi, mid, lo) of float64/float32 array."""
    a = np.asarray(a, np.float64)
    hi = a.astype(BF).astype(np.float64)
    mid = (a - hi).astype(BF).astype(np.float64)
    lo = (a - hi - mid).astype(BF)
    return hi.astype(BF), mid.astype(BF), lo


def _prep_inputs(inputs):
    """Host-side constant relayout + per-core slicing. Returns in_maps list."""
    xyz = _f32(inputs["xyz"])          # [2, 384, 3]
    feats = _f32(inputs["feats"])      # [2, 384, 1]

    Wq, Wk, Wv = inputs["tb_Wq"], inputs["tb_Wk"], inputs["tb_Wv"]
    Wg1, bg1 = inputs["tb_Wg1"], inputs["tb_bg1"]
    Wg2, bg2 = inputs["tb_Wg2"], inputs["tb_bg2"]
    Wpe, bpe = inputs["tb_Wpe"], inputs["tb_bpe"]

    L_G1P3 = np.zeros((NB, 96, 128), np.float32)
    L_Ps3 = np.zeros((NB, 96, 128), np.float32)
    L_nWk = np.zeros((NB, 128, 128), np.float32)
    L_Wv = np.zeros((NB, 128, 128), np.float32)
    L_G1Q = np.zeros((NB, 128, 128), np.float32)
    L_Wg1 = np.zeros((NB, 128, 128), np.float32)
    L_Wg2 = np.zeros((NB, 128, 128), np.float32)
    L_pd9 = np.zeros((3, 9 * 128), np.float32)
    rows2 = np.zeros((1, 896), np.float32)
    cols24 = np.zeros((128, 24), np.float32)
    for i in range(NB):
        Ws, Wd = _wpe_split(_f32(Wpe[i]))
        g1 = _f32(Wg1[i])
        G1P = (g1 @ Ws).T            # [30, 128]
        PsT = Ws.T
        for r in range(3):
            L_G1P3[i, 32 * r:32 * r + 30] = G1P
            L_Ps3[i, 32 * r:32 * r + 30] = PsT
        L_nWk[i] = (-_f32(Wk[i])).T
        L_Wv[i] = _f32(Wv[i]).T
        L_pd9[:, 128 * i:128 * (i + 1)] = (-4.0 * Wd).T
        L_G1Q[i] = (g1 @ _f32(Wq[i])).T
        L_pd9[:, 128 * (3 + i):128 * (4 + i)] = (4.0 * (g1 @ Wd)).T
        L_pd9[:, 128 * (6 + i):128 * (7 + i)] = (4.0 * Wd).T
        L_Wg1[i] = g1.T
        L_Wg2[i] = _f32(Wg2[i]).T
        rows2[0, 128 * i:128 * (i + 1)] = g1 @ _f32(bpe[i]) + _f32(bg1[i])
        rows2[0, 128 * (3 + i):128 * (4 + i)] = _f32(bpe[i])
        cols24[:, _CC["bg2"] + i] = _f32(bg2[i])
        cols24[:, _CC["gam"] + i] = _f32(inputs["tb_gamma"][i])
        cols24[:, _CC["bet"] + i] = _f32(inputs["tb_beta"][i])
    rows2[0, 768:896] = _f32(inputs["enc_W"])[:, 0]
    cols24[:, _CC["encb"]] = _f32(inputs["enc_b"])
    for j in range(NF):
        cols24[:, _CC["emb1"] + j] = _f32(inputs["em_b1"][j])
        cols24[:, _CC["emb2"] + j] = _f32(inputs["em_b2"][j])
        cols24[:, _CC["emg"] + j] = _f32(inputs["em_gamma"][j])
        cols24[:, _CC["embe"] + j] = _f32(inputs["em_beta"][j])
    cols24[:, _CC["f1b"]:_CC["f1b"] + 2] = \
        _f32(inputs["fcf_b1"]).reshape(2, 128).T
    cols24[:, _CC["f2b"]:_CC["f2b"] + 2] = \
        _f32(inputs["fcf_b2"]).reshape(2, 128).T

    W2 = _f32(inputs["fcf_W2"])
    L_f2 = np.zeros((2, 2, 128, 128), np.float32)
    for h in range(2):
        for k in range(2):
            L_f2[h, k] = W2.T[k * 128:(k + 1) * 128, h * 128:(h + 1) * 128]

    # fp8 half-identity pair for DoubleRow injection (scaled by 1/FP8_SCALE)
    eye = np.eye(128, dtype=np.float32) * (0.5 / FP8_SCALE)
    i128h8 = np.concatenate([eye, eye], axis=1).astype(
        ml_dtypes.float8_e4m3)

    # stage-1 phase constants: s_t = 4 * FREQS_t / (2*pi), 3-level bf16
    # split of SD/xk so the phase matmul runs in bf16 at fp32-like accuracy
    s_vals = (4.0 * FREQS / (2 * np.pi))         # [5] float64
    SD = np.zeros((3, 47), np.float64)
    for j in range(3):
        for t in range(5):
            SD[j, 5 * j + t] = s_vals[t]
            SD[j, 32 + 5 * j + t] = s_vals[t]
    Sa, Sb, Sc = _split3(SD)
    SD18 = np.concatenate([Sa, Sa, Sb, Sa, Sc, Sb], axis=0).astype(BF)

    com = {
        "feats_row": _bf(feats.reshape(1, 768)),
        "SD18": np.ascontiguousarray(SD18),
        "i128h8": np.ascontiguousarray(i128h8),
        "i128": _bf(np.eye(128, dtype=np.float32)),
        "L_G1P3": _bf(L_G1P3), "L_Ps3": _bf(L_Ps3),
        "L_Wg1": _bf(L_Wg1), "L_Wg2": _bf(L_Wg2),
        "L_nWk": _bf(L_nWk), "L_Wv": _bf(L_Wv), "L_G1Q": _bf(L_G1Q),
        "L_pd9": _bf(L_pd9), "rows2": _bf(rows2), "cols24": cols24,
        "L_em1": _bf(np.stack([_f32(inputs["em_W1"][j]).T for j in range(NF)])),
        "L_em2": _bf(np.stack([_f32(inputs["em_W2"][j]).T for j in range(NF)])),
        "L_f1": _bf(_f32(inputs["fcf_W1"]).T.reshape(128, 2, 128)
                    .transpose(1, 0, 2)),
        "L_f2": _bf(L_f2),
    }

    in_maps = []
    for c in range(8):
        b, qo = c // 4, (c % 4) * 96
        xk = xyz[b].T                      # [3, 384]
        # per-chunk lhsT [47, 96]: rows 0-14 pair with sin(2*pi*pf) and
        # carry cos(2*pi*of); rows 32-46 pair with cos(2*pi*pf) and carry
        # sin(2*pi*of); rows 15-31 are zero padding.
        L_OF2 = np.zeros((32, 47, 96), np.float64)
        for cch in range(32):
            for qr in range(3):
                qg = qo + 3 * cch + qr
                for j in range(3):
                    for t in range(10):
                        col = 32 * qr + 10 * j + t
                        ph = 5 * j + (t % 5)
                        off = (0.25 if t >= 5 else 0.0) \
                            - s_vals[t % 5] * np.float64(xyz[b, qg, j])
                        ang = 2 * np.pi * (off - np.round(off))
                        L_OF2[cch, ph, col] = np.cos(ang)
                        L_OF2[cch, 32 + ph, col] = np.sin(ang)
        m = dict(com)
        Xa, Xb, Xc = _split3(xk)
        m["xk18"] = np.ascontiguousarray(
            np.concatenate([Xa, Xb, Xa, Xc, Xa, Xb], axis=0).astype(BF))
        m["L_OF2"] = _bf(L_OF2.transpose(1, 0, 2).reshape(47, 32 * 96))
        m["xk_b"] = _bf(xk)
        m["xq_b"] = _bf(xk[:, qo:qo + 96])
        m["feats_q"] = _bf(feats[b, qo:qo + 96].reshape(1, 96))
        in_maps.append(m)
    return in_maps


def kernel(**inputs):
    from concourse.bass_utils import run_bass_kernel_spmd

    nc = _build()
    in_maps = _prep_inputs(inputs)
    res = run_bass_kernel_spmd(nc, in_maps, list(range(8)))
    return np.asarray(res.results[0]["out"], np.float32)


if __name__ == "__main__":
    print("smoke build only")


# revision 59
# speedup vs baseline: 1.0013x; 1.0013x over previous
"""NePuEncoder Bass/Tile kernel for 8 Trainium2 NeuronCores.

Sharding: query-parallel. Core c handles batch b=c//4, queries qo=(c%4)*96 ..
qo+96 of that batch. Channel-major layout [128 chan, keys] throughout.

v3 pipeline (software-pipelined 2 pairs ahead, per block):
  PE : hpre = G1P@trig + fp8-DoubleRow inject of EKW=Wg1@EK (precomputed/blk)
       logits = Wg2@h ; pos = Ps@trig + fp8-DoubleRow inject of VK
  ACT: one exp per 2-query logit pair (bias bg2 per-block) with accum_out
       T = S0(2k)+S0(2k+1); even-query relu + cols [0,CA) of odd relu
  DVE: S1 accumulation (stt), S0(2k) via tensor_reduce on the even w half
       (odd S0 recovered as T - S0even at the tail), odd-relu cols [CA,384)
Stage 1: pairwise trig by angle addition: frac(s*xk) key phases once, one
Sin builds [sin;cos](2*pi*pf); per chunk ONE bf16 matmul whose lhsT carries
cos/sin(2*pi*of_q) host-side, evictions alternate ACT/DVE. BN rsqrt uses
Ln+Exp under one activation table (natural_log_exp_and_others) loaded once.
"""
import sys

sys.path.insert(0, "/opt/trn_rl_repo")

import numpy as np
import ml_dtypes

B, N, D, DS, LAT, FD, NF = 2, 384, 128, 3, 256, 1, 2
NB = NF + 1
NQ = 96                  # queries per core
RELU_ACT_EVEN = True     # even queries relu on ACT, odd on DVE (per-pair AD)
FREQS = np.linspace(1.0, 32.0, 5).astype(np.float64)
EPS = 1e-5
TWO_PI = float(2 * np.pi)
C_ROUND = float(3 << 22)  # 2^23 + 2^22: fp32 round-to-nearest-even trick
FP8_INJ = True           # fp8 DoubleRow injection matmuls for EKW/VK
FP8_SCALE = 8.0
S0_SPLIT_CA = 178        # odd-relu columns on ACT; rest on DVE
EXP_PAIR = True          # one exp op per 2 queries

BF = ml_dtypes.bfloat16
F16 = np.float16

# cols24 column map (per-channel [128,1] fp32 constants)
_CC = {"bg2": 0, "gam": 3, "bet": 6, "emb1": 9, "emb2": 11, "emg": 13,
       "embe": 15, "encb": 17, "f1b": 18, "f2b": 20}


def _bf(x):
    return np.ascontiguousarray(np.asarray(x, np.float32).astype(BF))


def _f32(x):
    return np.ascontiguousarray(np.asarray(x, np.float32))


def _f16(x):
    return np.ascontiguousarray(np.asarray(x, np.float32).astype(F16))


def _wpe_split(Wpe):
    """W_s [128,30] trig cols (sin-sign absorbed for npd = xk - xq), W_d [128,3]."""
    Ws = np.zeros((D, 30), np.float32)
    for j in range(3):
        for t in range(10):
            r = 10 * j + t
            if t < 5:
                Ws[:, r] = -Wpe[:, 3 + 6 * t + j]
            else:
                Ws[:, r] = Wpe[:, 3 + 6 * (t - 5) + 3 + j]
    return Ws, Wpe[:, 0:3].astype(np.float32)


_CACHE = {}


def _build(variant="spmd"):
    if variant in _CACHE:
        return _CACHE[variant]

    import concourse.bacc as bacc
    import concourse.bass as bass
    import concourse.tile as tile
    from concourse import mybir

    F32, BF16 = mybir.dt.float32, mybir.dt.bfloat16
    FP16, FP8 = mybir.dt.float16, mybir.dt.float8e4
    AF = mybir.ActivationFunctionType
    OP = mybir.AluOpType
    DR = mybir.MatmulPerfMode.DoubleRow

    nc = bacc.Bacc(None, target_bir_lowering=False,
                   num_devices=(8 if variant == "spmd" else 1))

    def din(name, shape, dt=BF16):
        return nc.dram_tensor(name, shape, dt, kind="ExternalInput")

    # per-core inputs
    xk18 = din("xk18", [18, 384])
    L_OF2 = din("L_OF2", [47, 32 * 96])
    xk_b = din("xk_b", [3, 384])
    xq_b = din("xq_b", [3, 96])
    feats_q = din("feats_q", [1, 96])
    # replicated inputs
    feats_row = din("feats_row", [1, 768])
    SD18 = din("SD18", [18, 47])
    i128h8 = din("i128h8", [128, 256], FP8)
    i128 = din("i128", [128, 128])
    L_G1P3 = din("L_G1P3", [NB, 96, 128])
    L_Ps3 = din("L_Ps3", [NB, 96, 128])
    L_Wg1 = din("L_Wg1", [NB, 128, 128])
    L_Wg2 = din("L_Wg2", [NB, 128, 128])
    L_nWk = din("L_nWk", [NB, 128, 128])
    L_Wv = din("L_Wv", [NB, 128, 128])
    L_G1Q = din("L_G1Q", [NB, 128, 128])
    L_pd9 = din("L_pd9", [3, 9 * 128])     # nPd4(3) | G1Pd4(3) | Pd4(3)
    rows2 = din("rows2", [1, 896])         # c1(3) | bpe(3) | enc, row vectors
    cols24 = din("cols24", [128, 24], F32)
    L_em1 = din("L_em1", [NF, 128, 128])
    L_em2 = din("L_em2", [NF, 128, 128])
    L_f1 = din("L_f1", [2, 128, 128])
    L_f2 = din("L_f2", [2, 2, 128, 128])

    out_d = nc.dram_tensor("out", [2, 256], F32, kind="ExternalOutput")
    RG = [[0, 1, 2, 3, 4, 5, 6, 7]]

    with tile.TileContext(nc) as tc:
        with (
            tc.tile_pool(name="sing", bufs=1) as sing,
            tc.tile_pool(name="fpool", bufs=2) as fpool,
            tc.tile_pool(name="blk", bufs=2) as blk,
            tc.tile_pool(name="hp", bufs=6) as hp,
            tc.tile_pool(name="wp", bufs=2) as wp,
            tc.tile_pool(name="wide", bufs=2) as wide,
            tc.tile_pool(name="smalls", bufs=4) as smalls,
            tc.tile_pool(name="dram", bufs=1, space="DRAM") as dram,
        ):
            _dmaq = [nc.sync, nc.gpsimd]
            _qi = [0]

            def _q():
                e = _dmaq[_qi[0] % len(_dmaq)]
                _qi[0] += 1
                return e

            def load(src, shape, dt=BF16, pool=sing, tag=None):
                t = pool.tile(shape, dt, tag=tag, name=tag or "ld")
                _q().dma_start(out=t, in_=src)
                return t

            def loadfam(srcT, nblk, tag, np_=128):
                t = sing.tile([np_, nblk * 128], BF16, tag=tag, name=tag)
                ap = srcT[:]
                s = bass.AP(tensor=ap.tensor, offset=ap.offset,
                            ap=[[128, np_], [np_ * 128, nblk], [1, 128]])
                _q().dma_start(out=t.rearrange("p (i c) -> p i c", i=nblk), in_=s)
                return [t[:, i * 128:(i + 1) * 128] for i in range(nblk)]

            # stage-1 critical loads first
            xkf_sb = load(xk18[:], [18, 384], tag="xkf")
            sd_sb = load(SD18[:], [18, 47], tag="sd")
            lof_sb = load(L_OF2[:], [47, 32 * 96], tag="lof")

            def load_weights():
                o = {}
                o["xkb"] = load(xk_b[:], [3, 384], tag="sxkb")
                o["xqb"] = load(xq_b[:], [3, 96], tag="sxqb")
                o["featsq"] = load(feats_q[:], [1, 96], tag="sfq")
                o["feats"] = load(feats_row[:], [1, 768], tag="sfr")
                o["i128h8"] = load(i128h8[:], [128, 256], FP8, tag="i128h8")
                if not FP8_INJ:
                    o["i128"] = load(i128[:], [128, 128], tag="si128")
                o["G1P"] = loadfam(L_G1P3, NB, "g1p3", np_=96)
                o["Ps"] = loadfam(L_Ps3, NB, "ps3", np_=96)
                o["Wg1"] = loadfam(L_Wg1, NB, "wg1")
                o["Wg2"] = loadfam(L_Wg2, NB, "wg2")
                o["nWk"] = loadfam(L_nWk, NB, "nwk")
                o["Wv"] = loadfam(L_Wv, NB, "wv")
                o["G1Q"] = loadfam(L_G1Q, NB, "g1q")
                pd9 = load(L_pd9[:], [3, 9 * 128], tag="pd9")
                o["nPd4"] = [pd9[:, 128 * i:128 * (i + 1)] for i in range(3)]
                o["G1Pd4"] = [pd9[:, 128 * (3 + i):128 * (4 + i)] for i in range(3)]
                o["Pd4"] = [pd9[:, 128 * (6 + i):128 * (7 + i)] for i in range(3)]
                r2 = load(rows2[:], [1, 896], tag="rows2")
                o["c1"] = [r2[0:1, 128 * i:128 * (i + 1)] for i in range(3)]
                o["bpe"] = [r2[0:1, 128 * (3 + i):128 * (4 + i)] for i in range(3)]
                o["enc"] = r2[0:1, 768:896]
                c24 = load(cols24[:], [128, 24], F32, tag="cols24")
                cc = lambda k, i=0: c24[:, _CC[k] + i:_CC[k] + i + 1]
                o["bg2"] = [cc("bg2", i) for i in range(3)]
                o["gam"] = [cc("gam", i) for i in range(3)]
                o["bet"] = [cc("bet", i) for i in range(3)]
                o["emb1"] = [cc("emb1", j) for j in range(NF)]
                o["emb2"] = [cc("emb2", j) for j in range(NF)]
                o["emg"] = [cc("emg", j) for j in range(NF)]
                o["embe"] = [cc("embe", j) for j in range(NF)]
                o["encb"] = cc("encb")
                o["f1b"] = [cc("f1b", h) for h in range(2)]
                o["f2b"] = [cc("f2b", h) for h in range(2)]
                o["em1"] = loadfam(L_em1, NF, "em1")
                o["em2"] = loadfam(L_em2, NF, "em2")
                o["f1"] = loadfam(L_f1, 2, "f1")
                f2t = loadfam(L_f2, 4, "f2")
                o["f2"] = [[f2t[2 * h + k] for k in range(2)] for h in range(2)]
                return o

            ones96 = sing.tile([1, 96], BF16, tag="ones96")
            nc.vector.memset(ones96, 1.0)
            eps_t = sing.tile([128, 1], F32, tag="epst")
            nc.vector.memset(eps_t, EPS)
            dumA = sing.tile([128, 1], BF16, tag="dumA")
            dumP = sing.tile([128, 1], BF16, tag="dumP")

            trig_all = sing.tile([96, 32 * 384], BF16, tag="trigall",
                                 name="trig_all")

            W = load_weights()

            # ---------- stage 1: separable pairwise trig -------------------
            # key phases p[(5j+t),k] = s_t * xk[j,k]; pf = p - round(p);
            # sc = [sin(2*pi*pf); cos(2*pi*pf)] once.  Per chunk the
            # per-query offset trig is folded into the lhsT (host), so
            # trig = sin(2*pi*(pf+of)) comes straight out of one bf16
            # matmul via angle addition; evict to bf16 alternates ACT/DVE.
            with (
                tc.tile_pool(name="s1aux", bufs=1) as s1aux,
                tc.tile_pool(name="s1ps", bufs=4, space="PSUM") as s1ps,
            ):
                pA = s1ps.tile([128, 2, 512], F32, tag="r2", name="pA")
                nc.tensor.matmul(pA[0:47, 0, 0:384], sd_sb[:, :], xkf_sb,
                                 start=True, stop=True)
                nP = s1aux.tile([47, 384], F32, tag="nP")
                nc.vector.tensor_scalar(out=nP, in0=pA[0:47, 0, 0:384],
                                        scalar1=C_ROUND, scalar2=C_ROUND,
                                        op0=OP.add, op1=OP.subtract)
                pf_t = s1aux.tile([47, 384], F32, tag="pf")
                nc.vector.scalar_tensor_tensor(
                    out=pf_t, in0=pA[0:47, 0, 0:384], scalar=1.0,
                    in1=nP, op0=OP.mult, op1=OP.subtract)
                b47 = s1aux.tile([47, 1], F32, tag="b47")
                nc.vector.memset(b47, 0.0)
                nc.vector.memset(b47[32:47, :], float(np.pi / 2))
                sc_t = s1aux.tile([47, 384], BF16, tag="sc")
                nc.scalar.activation(out=sc_t, in_=pf_t, func=AF.Sin,
                                     bias=b47, scale=TWO_PI)

                f_full = fpool.tile([128, 768], BF16, tag="ffull")
                fq = fpool.tile([128, 96], BF16, tag="fq")
                for g in range(16):
                    rt = s1ps.tile([128, 2, 512], F32, tag="r2", name="rt")
                    for u in range(2):
                        c = 2 * g + u
                        nc.tensor.matmul(rt[0:96, u, 0:384],
                                         lof_sb[:, 96 * c:96 * (c + 1)],
                                         sc_t[:, :], start=True, stop=True)
                    ts = trig_all[:, 768 * g:768 * (g + 1)]
                    ts3 = ts.rearrange("p (u k) -> p u k", u=2)
                    if g % 2 == 0:
                        nc.scalar.activation(out=ts3, in_=rt[0:96, :, 0:384],
                                             func=AF.Copy, bias=0.0, scale=1.0)
                    else:
                        nc.vector.tensor_scalar(out=ts3,
                                                in0=rt[0:96, :, 0:384],
                                                scalar1=1.0, scalar2=None,
                                                op0=OP.mult)
                    if g == 10:
                        # feature init interleaved into stage-1 ACT slack
                        fi = s1ps.tile([128, 2, 512], F32, tag="r2", name="fi")
                        for half in range(2):
                            nc.tensor.matmul(
                                fi[:, half, 0:384], W['enc'],
                                W['feats'][:, half * 384:(half + 1) * 384],
                                start=True, stop=True)
                            nc.scalar.activation(
                                out=f_full[:, half * 384:(half + 1) * 384],
                                in_=fi[:, half, 0:384], func=AF.Identity,
                                bias=W['encb'], scale=1.0)
                        fi2 = s1ps.tile([128, 2, 512], F32, tag="r2", name="fi2")
                        nc.tensor.matmul(fi2[:, 0, 0:96], W['enc'],
                                         W['featsq'], start=True, stop=True)
                        nc.scalar.activation(out=fq, in_=fi2[:, 0, 0:96],
                                             func=AF.Identity, bias=W['encb'],
                                             scale=1.0)
            tc.no_sync_barrier()

            # lock the ACT table to the exp+ln set for the rest of the kernel
            try:
                from concourse.hw_specs import get_activation_tables
                _tabs = list(get_activation_tables(nc.m.arch))
                _idx = _tabs.index("natural_log_exp_and_others")
                nc.scalar.add_instruction(mybir.InstLoadActFuncSet(
                    name=nc.scalar.bass.get_next_instruction_name(),
                    act_func_set_id=_idx, ins=[], outs=[]))
            except Exception:
                pass

            def tsl(m):
                r = m % 3
                return trig_all[32 * r:32 * r + 30,
                                384 * (m // 3):384 * (m // 3 + 1)]

            with (
                tc.tile_pool(name="ps_a", bufs=4, space="PSUM") as ps_a,
                tc.tile_pool(name="ps_b", bufs=2, space="PSUM") as ps_b,
                tc.tile_pool(name="ps_g", bufs=1, space="PSUM") as ps_g,
            ):
                pid = nc.scalar.partition_id()
                fb = fpool.tile([128, 384], BF16, tag="fb")
                with tc.If(pid < 4) as cmp:
                    nc.scalar.copy(fb, f_full[:, 0:384])
                with cmp.Else():
                    nc.scalar.copy(fb, f_full[:, 384:768])

                def affine_evict(src_ap, sc, b2, shape, dt=BF16, tag="aff",
                                 pool=None):
                    t = (pool or fpool).tile(shape, dt, tag=tag)
                    nc.scalar.activation(out=t, in_=src_ap, func=AF.Identity,
                                         bias=b2, scale=sc)
                    return t

                # ---------- transformer blocks ----------
                for i in range(NB):
                    # block consts
                    pa = ps_a.tile([128, 512], F32, tag="pa")
                    nc.tensor.matmul(pa[:, 0:384], W['nWk'][i], fb, start=True,
                                     stop=False)
                    nc.tensor.matmul(pa[:, 0:384], W['nPd4'][i], W['xkb'],
                                     start=False, stop=True)
                    EK = blk.tile([128, 384], BF16, tag="EK")
                    nc.scalar.copy(EK, pa[:, 0:384])

                    if FP8_INJ:
                        pa = ps_a.tile([128, 512], F32, tag="pa")
                        nc.tensor.matmul(pa[:, 0:384], W['Wg1'][i], EK,
                                         start=True, stop=True)
                        ekw8 = blk.tile([128, 2, 384], FP8, tag="ekw8")
                        nc.scalar.activation(
                            out=ekw8,
                            in_=pa[:, 0:384].rearrange("p (o k) -> p o k", o=1)
                                .broadcast_to((128, 2, 384)),
                            func=AF.Copy, bias=0.0, scale=FP8_SCALE)

                    pb = ps_b.tile([128, 512], F32, tag="pb")
                    nc.tensor.matmul(pb[:, 0:384], W['Wv'][i], fb, start=True,
                                     stop=False)
                    nc.tensor.matmul(pb[:, 0:384], W['nPd4'][i], W['xkb'],
                                     start=False, stop=True)
                    if FP8_INJ:
                        vk8 = blk.tile([128, 2, 384], FP8, tag="vk8")
                        nc.scalar.activation(
                            out=vk8,
                            in_=pb[:, 0:384].rearrange("p (o k) -> p o k", o=1)
                                .broadcast_to((128, 2, 384)),
                            func=AF.Copy, bias=0.0, scale=FP8_SCALE)
                        VK = None
                    else:
                        VK = blk.tile([128, 384], BF16, tag="VK")
                        nc.scalar.copy(VK, pb[:, 0:384])

                    pa = ps_a.tile([128, 512], F32, tag="pa")
                    nc.tensor.matmul(pa[:, 0:96], W['G1Q'][i], fq, start=True,
                                     stop=False)
                    nc.tensor.matmul(pa[:, 0:96], W['G1Pd4'][i], W['xqb'],
                                     start=False, stop=False)
                    nc.tensor.matmul(pa[:, 0:96], W['c1'][i], ones96,
                                     start=False, stop=True)
                    QB = blk.tile([128, 96], F32, tag="QB")
                    nc.scalar.copy(QB, pa[:, 0:96])

                    pb = ps_b.tile([128, 512], F32, tag="pb")
                    nc.tensor.matmul(pb[:, 0:96], W['Pd4'][i], W['xqb'],
                                     start=True, stop=False)
                    nc.tensor.matmul(pb[:, 0:96], W['bpe'][i], ones96,
                                     start=False, stop=True)
                    QP = blk.tile([128, 96], F32, tag="QP")
                    nc.scalar.copy(QP, pb[:, 0:96])

                    S1 = blk.tile([128, 96], F32, tag="S1")
                    S0e = blk.tile([128, 48], F32, tag="S0e")
                    Tp = blk.tile([128, 48], F32, tag="Tp")
                    S0 = blk.tile([128, 96], F32, tag="S0")
                    R = blk.tile([128, 96], F32, tag="R")
                    RES = blk.tile([128, 96], F32, tag="RES")
                    payload = blk.tile([128, 128], F32, tag="payload")
                    nc.vector.memset(payload[:, 96:128], 0.0)

                    h_t = [None, None, None, None]  # h tiles mod 4

                    def mm_h(m):
                        hpre = ps_a.tile([128, 512], F32, tag="pa", name="hpre")
                        nc.tensor.matmul(hpre[:, 0:384],
                                         W['G1P'][i][32 * (m % 3):32 * (m % 3) + 30, :],
                                         tsl(m), start=True, stop=False)
                        if FP8_INJ:
                            nc.tensor.matmul(
                                hpre[:, 0:384],
                                W['i128h8'][:, :].rearrange("p (b f) -> p b f", b=2),
                                ekw8[:, :, :], start=False, stop=True,
                                perf_mode=DR, skip_group_check=True)
                        else:
                            nc.tensor.matmul(hpre[:, 0:384], W['Wg1'][i], EK,
                                             start=False, stop=True)
                        return hpre

                    CA = S0_SPLIT_CA

                    def relu_h(m, hpre):
                        t = hp.tile([128, 384], BF16, tag="h", name="h_t")
                        if m % 2 == 0:
                            nc.scalar.activation(out=t, in_=hpre[:, 0:384],
                                                 func=AF.Relu,
                                                 bias=QB[:, m:m + 1], scale=1.0)
                        else:
                            nc.scalar.activation(out=t[:, 0:CA],
                                                 in_=hpre[:, 0:CA],
                                                 func=AF.Relu,
                                                 bias=QB[:, m:m + 1], scale=1.0)
                        h_t[m % 4] = t

                    def relu_h_dve(m, hpre):
                        t = h_t[m % 4]
                        nc.vector.tensor_scalar(
                            out=t[:, CA:384], in0=hpre[:, CA:384],
                            scalar1=QB[:, m:m + 1], scalar2=0.0,
                            op0=OP.add, op1=OP.max)

                    def mm_tail(m, w_ap):
                        pos = ps_b.tile([128, 512], F32, tag="pb", name="pos")
                        nc.tensor.matmul(pos[:, 0:384],
                                         W['Ps'][i][32 * (m % 3):32 * (m % 3) + 30, :],
                                         tsl(m), start=True, stop=False)
                        if FP8_INJ:
                            nc.tensor.matmul(
                                pos[:, 0:384],
                                W['i128h8'][:, :].rearrange("p (b f) -> p b f", b=2),
                                vk8[:, :, :], start=False, stop=True,
                                perf_mode=DR, skip_group_check=True)
                        else:
                            nc.tensor.matmul(pos[:, 0:384], W['i128'], VK,
                                             start=False, stop=True)
                        nc.vector.scalar_tensor_tensor(
                            out=dumA.broadcast_to((128, 384)),
                            in0=pos[:, 0:384], scalar=QP[:, m:m + 1],
                            in1=w_ap, op0=OP.add, op1=OP.mult,
                            accum_out=S1[:, m:m + 1])

                    # software pipeline, 2 pairs ahead: hpre matmuls for
                    # pair kp+2 issue at iter kp; relus follow this iter's exp
                    # so the exp->relu->logits->exp cycle never serializes.
                    for m0 in range(4):
                        hpre0 = mm_h(m0)
                        relu_h(m0, hpre0)
                        if m0 % 2 == 1:
                            relu_h_dve(m0, hpre0)
                    st = smalls.tile([128, 2, 6], F32, tag="bnst2")
                    for kp in range(NQ // 2):
                        lg = ps_g.tile([128, 2, 512], F32, tag="lg", name="lg")
                        nc.tensor.matmul(lg[:, 0, 0:384], W['Wg2'][i],
                                         h_t[(2 * kp) % 4], start=True, stop=True)
                        nc.tensor.matmul(lg[:, 1, 0:384], W['Wg2'][i],
                                         h_t[(2 * kp + 1) % 4], start=True,
                                         stop=True)
                        if kp < NQ // 2 - 2:
                            hA = mm_h(2 * kp + 4)
                            hB = mm_h(2 * kp + 5)
                        w2 = wp.tile([128, 2, 384], BF16, tag="w2", name="w2")
                        nc.scalar.activation(out=w2, in_=lg[:, :, 0:384],
                                             func=AF.Exp, bias=W['bg2'][i],
                                             scale=1.0,
                                             accum_out=Tp[:, kp:kp + 1])
                        if kp < NQ // 2 - 2:
                            relu_h(2 * kp + 4, hA)
                            relu_h(2 * kp + 5, hB)
                        mm_tail(2 * kp, w2[:, 0, :])
                        mm_tail(2 * kp + 1, w2[:, 1, :])
                        nc.vector.tensor_reduce(out=S0e[:, kp:kp + 1],
                                                in_=w2[:, 0, :],
                                                axis=mybir.AxisListType.X,
                                                op=OP.add)
                        if kp < NQ // 2 - 2:
                            relu_h_dve(2 * kp + 5, hB)
                        if kp == 28:
                            # first-half tail math (queries 0-47 complete)
                            sv = S0[:, 0:48].rearrange("p (q t) -> p q t", t=2)
                            nc.vector.tensor_copy(sv[:, :, 0], S0e[:, 0:24])
                            nc.vector.tensor_tensor(out=sv[:, :, 1],
                                                    in0=Tp[:, 0:24],
                                                    in1=S0e[:, 0:24],
                                                    op=OP.subtract)
                            nc.vector.reciprocal(out=R[:, 0:48],
                                                 in_=S0[:, 0:48])
                            nc.vector.tensor_tensor(out=RES[:, 0:48],
                                                    in0=S1[:, 0:48],
                                                    in1=R[:, 0:48], op=OP.mult)
                            nc.vector.tensor_tensor(out=payload[:, 0:48],
                                                    in0=RES[:, 0:48],
                                                    in1=fq[:, 0:48], op=OP.add)
                            nc.vector.bn_stats(out=st[:, 0, :],
                                               in_=payload[:, 0:48])

                    # block tail: second-half o = S1/S0 + fq; stats; gather
                    sv2 = S0[:, 48:96].rearrange("p (q t) -> p q t", t=2)
                    nc.vector.tensor_copy(sv2[:, :, 0], S0e[:, 24:48])
                    nc.vector.tensor_tensor(out=sv2[:, :, 1],
                                            in0=Tp[:, 24:48],
                                            in1=S0e[:, 24:48], op=OP.subtract)
                    nc.vector.reciprocal(out=R[:, 48:96], in_=S0[:, 48:96])
                    nc.vector.tensor_tensor(out=RES[:, 48:96],
                                            in0=S1[:, 48:96], in1=R[:, 48:96],
                                            op=OP.mult)
                    nc.vector.tensor_tensor(out=payload[:, 48:96],
                                            in0=RES[:, 48:96], in1=fq[:, 48:96],
                                            op=OP.add)
                    nc.vector.bn_stats(out=st[:, 1, :], in_=payload[:, 48:96])
                    mv = smalls.tile([128, 2], F32, tag="bnmv")
                    nc.vector.bn_aggr(out=mv, in_=st)
                    nc.vector.tensor_copy(payload[:, 96:97], mv[:, 0:1])
                    msq = smalls.tile([128, 1], F32, tag="msq")
                    nc.vector.tensor_tensor(out=msq, in0=mv[:, 0:1],
                                            in1=mv[:, 0:1], op=OP.mult)
                    nc.vector.tensor_tensor(out=payload[:, 97:98],
                                            in0=mv[:, 1:2], in1=msq, op=OP.add)

                    ag_in = dram.tile([128, 128], F32, tag=f"agin{i}")
                    if variant == "spmd":
                        ag_out = dram.tile([8, 128, 128], F32,
                                           addr_space="Shared", tag=f"agout{i}")
                    else:
                        ag_out = dram.tile([8, 128, 128], F32, tag=f"agout{i}")
                    ago_ap = ag_out[:]
                    if variant == "spmd":
                        nc.gpsimd.dma_start(out=ag_in, in_=payload)
                        nc.gpsimd.collective_compute(
                            "AllGather", OP.bypass, replica_groups=RG,
                            ins=[ag_in[:].opt()], outs=[ag_out[:].opt()])
                    else:
                        bsrc = payload[:, :].rearrange("p (o k) -> p o k", o=1)                             .broadcast_to((128, 8, 128))
                        bdst = bass.AP(tensor=ago_ap.tensor, offset=ago_ap.offset,
                                       ap=[[128, 128], [128 * 128, 8], [1, 128]])
                        nc.sync.dma_start(out=bdst, in_=bsrc)

                    gath = wide.tile([128, 8, 128], F32, tag="gath")
                    src = bass.AP(tensor=ago_ap.tensor, offset=ago_ap.offset,
                                  ap=[[128, 128], [128 * 128, 8], [1, 128]])
                    nc.sync.dma_start(out=gath, in_=src)


                    mg = smalls.tile([128, 1], F32, tag="mg")
                    nc.vector.tensor_reduce(out=mg, in_=gath[:, :, 96],
                                            axis=mybir.AxisListType.X, op=OP.add)
                    nc.vector.tensor_scalar(out=mg, in0=mg, scalar1=0.125,
                                            scalar2=None, op0=OP.mult)
                    e2g = smalls.tile([128, 1], F32, tag="e2g")
                    nc.vector.tensor_reduce(out=e2g, in_=gath[:, :, 97],
                                            axis=mybir.AxisListType.X, op=OP.add)
                    nc.vector.tensor_scalar(out=e2g, in0=e2g, scalar1=0.125,
                                            scalar2=None, op0=OP.mult)
                    var = smalls.tile([128, 1], F32, tag="var")
                    nc.vector.tensor_tensor(out=var, in0=mg, in1=mg, op=OP.mult)
                    nc.vector.tensor_tensor(out=var, in0=e2g, in1=var,
                                            op=OP.subtract)
                    lnv = smalls.tile([128, 1], F32, tag="lnv")
                    nc.scalar.activation(out=lnv, in_=var, func=AF.Ln,
                                         bias=eps_t, scale=1.0)
                    rs = smalls.tile([128, 1], F32, tag="rs")
                    nc.scalar.activation(out=rs, in_=lnv, func=AF.Exp, bias=0.0,
                                         scale=-0.5)
                    sc = smalls.tile([128, 1], F32, tag="sc")
                    nc.vector.tensor_tensor(out=sc, in0=W['gam'][i], in1=rs,
                                            op=OP.mult)
                    b2 = smalls.tile([128, 1], F32, tag="b2")
                    nc.vector.tensor_scalar(out=b2, in0=mg, scalar1=sc,
                                            scalar2=None, op0=OP.mult)
                    nc.vector.tensor_tensor(out=b2, in0=W['bet'][i], in1=b2,
                                            op=OP.subtract)

                    if i > 0:
                        f_full = fpool.tile([128, 768], BF16, tag="ffull")
                        nc.scalar.activation(
                            out=f_full[:, 0:384].rearrange(
                                "p (c k) -> p c k", c=4),
                            in_=gath[:, 0:4, 0:96], func=AF.Identity, bias=b2,
                            scale=sc)
                        nc.vector.scalar_tensor_tensor(
                            out=f_full[:, 384:768].rearrange(
                                "p (c k) -> p c k", c=4),
                            in0=gath[:, 4:8, 0:96], scalar=sc,
                            in1=b2.broadcast_to((128, 4, 96)),
                            op0=OP.mult, op1=OP.add)
                    if i < NB - 1:
                        fq = fpool.tile([128, 96], BF16, tag="fq")
                        nc.vector.tensor_scalar(out=fq, in0=payload[:, 0:96],
                                                scalar1=sc, scalar2=b2,
                                                op0=OP.mult, op1=OP.add)
                    if i == 0:
                        fb = fpool.tile([128, 384], BF16, tag="fb")
                        with tc.If(pid < 4) as cmp:
                            nc.scalar.activation(
                                out=fb.rearrange("p (c k) -> p c k", c=4),
                                in_=gath[:, 0:4, 0:96], func=AF.Identity,
                                bias=b2, scale=sc)
                        with cmp.Else():
                            nc.scalar.activation(
                                out=fb.rearrange("p (c k) -> p c k", c=4),
                                in_=gath[:, 4:8, 0:96], func=AF.Identity,
                                bias=b2, scale=sc)

                    # ---------- MLP ----------
                    if i > 0:
                        j = i - 1

                        def mlp_layer(lw, bias_ap, xin, width, tag):
                            t = wide.tile([128, width], BF16, tag=tag)
                            for h0 in range(0, width, 384):
                                wdt = min(384, width - h0)
                                pp = ps_a.tile([128, 512], F32, tag="pa")
                                nc.tensor.matmul(pp[:, 0:wdt], lw,
                                                 xin[:, h0:h0 + wdt],
                                                 start=True, stop=True)
                                if h0 == 0:
                                    nc.scalar.activation(out=t[:, h0:h0 + wdt],
                                                         in_=pp[:, 0:wdt],
                                                         func=AF.Relu,
                                                         bias=bias_ap,
                                                         scale=1.0)
                                else:
                                    nc.vector.tensor_scalar(
                                        out=t[:, h0:h0 + wdt],
                                        in0=pp[:, 0:wdt], scalar1=bias_ap,
                                        scalar2=0.0, op0=OP.add, op1=OP.max)
                            return t

                        y1f = mlp_layer(W['em1'][j], W['emb1'][j], f_full, 768,
                                        "y1f")
                        y2f = mlp_layer(W['em2'][j], W['emb2'][j], y1f, 768,
                                        "y2f")
                        o2f = wide.tile([128, 768], F32, tag="o2f")
                        nc.vector.tensor_tensor(out=o2f, in0=f_full, in1=y2f,
                                                op=OP.add)
                        if i < NB - 1:
                            y1q = mlp_layer(W['em1'][j], W['emb1'][j], fq, 96,
                                            "y1q")
                            y2q = mlp_layer(W['em2'][j], W['emb2'][j], y1q, 96,
                                            "y2q")
                            o2q = wide.tile([128, 96], F32, tag="o2q")
                            nc.vector.tensor_tensor(out=o2q, in0=fq, in1=y2q,
                                                    op=OP.add)

                        st2 = smalls.tile([128, 2, 6], F32, tag="st2")
                        nc.vector.bn_stats(out=st2[:, 0, :], in_=o2f[:, 0:384])
                        nc.vector.bn_stats(out=st2[:, 1, :], in_=o2f[:, 384:768])
                        mv2 = smalls.tile([128, 2], F32, tag="mv2")
                        nc.vector.bn_aggr(out=mv2, in_=st2)
                        lnv2 = smalls.tile([128, 1], F32, tag="lnv")
                        nc.scalar.activation(out=lnv2, in_=mv2[:, 1:2],
                                             func=AF.Ln, bias=eps_t, scale=1.0)
                        rs2 = smalls.tile([128, 1], F32, tag="rs")
                        nc.scalar.activation(out=rs2, in_=lnv2, func=AF.Exp,
                                             bias=0.0, scale=-0.5)
                        sc2 = smalls.tile([128, 1], F32, tag="sc")
                        nc.vector.tensor_tensor(out=sc2, in0=W['emg'][j],
                                                in1=rs2, op=OP.mult)
                        b22 = smalls.tile([128, 1], F32, tag="b2")
                        nc.vector.tensor_scalar(out=b22, in0=mv2[:, 0:1],
                                                scalar1=sc2, scalar2=None,
                                                op0=OP.mult)
                        nc.vector.tensor_tensor(out=b22, in0=W['embe'][j],
                                                in1=b22, op=OP.subtract)
                        if i == NB - 1:
                            f_full = affine_evict(o2f[:], sc2, b22, [128, 768],
                                                  tag="ffull")
                        if i < NB - 1:
                            fb = fpool.tile([128, 384], BF16, tag="fb")
                            with tc.If(pid < 4) as cmp:
                                nc.scalar.activation(out=fb, in_=o2f[:, 0:384],
                                                     func=AF.Identity,
                                                     bias=b22, scale=sc2)
                            with cmp.Else():
                                nc.scalar.activation(out=fb,
                                                     in_=o2f[:, 384:768],
                                                     func=AF.Identity,
                                                     bias=b22, scale=sc2)
                            fq = fpool.tile([128, 96], BF16, tag="fq")
                            nc.vector.tensor_scalar(out=fq, in0=o2q,
                                                    scalar1=sc2, scalar2=b22,
                                                    op0=OP.mult, op1=OP.add)

                # ---------- final FC + max ----------
                ot = smalls.tile([128, 4], F32, tag="ot")
                for bb in range(2):
                    fbb = f_full[:, bb * 384:(bb + 1) * 384]
                    e1 = []
                    for h in range(2):
                        pp = ps_a.tile([128, 512], F32, tag="pa")
                        nc.tensor.matmul(pp[:, 0:384], W['f1'][h], fbb,
                                         start=True, stop=True)
                        e1t = wide.tile([128, 384], BF16, tag=f"e1{h}")
                        nc.scalar.activation(out=e1t, in_=pp[:, 0:384],
                                             func=AF.Relu, bias=W['f1b'][h],
                                             scale=1.0)
                        e1.append(e1t)
                    for h in range(2):
                        pp = ps_b.tile([128, 512], F32, tag="pb")
                        nc.tensor.matmul(pp[:, 0:384], W['f2'][h][0], e1[0],
                                         start=True, stop=False)
                        nc.tensor.matmul(pp[:, 0:384], W['f2'][h][1], e1[1],
                                         start=False, stop=True)
                        mx = smalls.tile([128, 1], F32, tag="mx")
                        nc.vector.tensor_reduce(out=mx, in_=pp[:, 0:384],
                                                axis=mybir.AxisListType.X,
                                                op=OP.max)
                        nc.vector.tensor_scalar(out=ot[:, 2 * bb + h:2 * bb + h + 1],
                                                in0=mx, scalar1=W['f2b'][h],
                                                scalar2=None, op0=OP.add)
                od_ap = out_d[:]
                dst = bass.AP(tensor=od_ap.tensor, offset=od_ap.offset,
                              ap=[[1, 128], [256, 2], [128, 2]])
                nc.sync.dma_start(out=dst, in_=ot)

    nc.compile()
    _CACHE[variant] = nc
    return nc


def _split3(a):
    """3-level bf16 split (hi, mid, lo) of float64/float32 array."""
    a = np.asarray(a, np.float64)
    hi = a.astype(BF).astype(np.float64)
    mid = (a - hi).astype(BF).astype(np.float64)
    lo = (a - hi - mid).astype(BF)
    return hi.astype(BF), mid.astype(BF), lo


def _prep_inputs(inputs):
    """Host-side constant relayout + per-core slicing. Returns in_maps list."""
    xyz = _f32(inputs["xyz"])          # [2, 384, 3]
    feats = _f32(inputs["feats"])      # [2, 384, 1]

    Wq, Wk, Wv = inputs["tb_Wq"], inputs["tb_Wk"], inputs["tb_Wv"]
    Wg1, bg1 = inputs["tb_Wg1"], inputs["tb_bg1"]
    Wg2, bg2 = inputs["tb_Wg2"], inputs["tb_bg2"]
    Wpe, bpe = inputs["tb_Wpe"], inputs["tb_bpe"]

    L_G1P3 = np.zeros((NB, 96, 128), np.float32)
    L_Ps3 = np.zeros((NB, 96, 128), np.float32)
    L_nWk = np.zeros((NB, 128, 128), np.float32)
    L_Wv = np.zeros((NB, 128, 128), np.float32)
    L_G1Q = np.zeros((NB, 128, 128), np.float32)
    L_Wg1 = np.zeros((NB, 128, 128), np.float32)
    L_Wg2 = np.zeros((NB, 128, 128), np.float32)
    L_pd9 = np.zeros((3, 9 * 128), np.float32)
    rows2 = np.zeros((1, 896), np.float32)
    cols24 = np.zeros((128, 24), np.float32)
    for i in range(NB):
        Ws, Wd = _wpe_split(_f32(Wpe[i]))
        g1 = _f32(Wg1[i])
        G1P = (g1 @ Ws).T            # [30, 128]
        PsT = Ws.T
        for r in range(3):
            L_G1P3[i, 32 * r:32 * r + 30] = G1P
            L_Ps3[i, 32 * r:32 * r + 30] = PsT
        L_nWk[i] = (-_f32(Wk[i])).T
        L_Wv[i] = _f32(Wv[i]).T
        L_pd9[:, 128 * i:128 * (i + 1)] = (-4.0 * Wd).T
        L_G1Q[i] = (g1 @ _f32(Wq[i])).T
        L_pd9[:, 128 * (3 + i):128 * (4 + i)] = (4.0 * (g1 @ Wd)).T
        L_pd9[:, 128 * (6 + i):128 * (7 + i)] = (4.0 * Wd).T
        L_Wg1[i] = g1.T
        L_Wg2[i] = _f32(Wg2[i]).T
        rows2[0, 128 * i:128 * (i + 1)] = g1 @ _f32(bpe[i]) + _f32(bg1[i])
        rows2[0, 128 * (3 + i):128 * (4 + i)] = _f32(bpe[i])
        cols24[:, _CC["bg2"] + i] = _f32(bg2[i])
        cols24[:, _CC["gam"] + i] = _f32(inputs["tb_gamma"][i])
        cols24[:, _CC["bet"] + i] = _f32(inputs["tb_beta"][i])
    rows2[0, 768:896] = _f32(inputs["enc_W"])[:, 0]
    cols24[:, _CC["encb"]] = _f32(inputs["enc_b"])
    for j in range(NF):
        cols24[:, _CC["emb1"] + j] = _f32(inputs["em_b1"][j])
        cols24[:, _CC["emb2"] + j] = _f32(inputs["em_b2"][j])
        cols24[:, _CC["emg"] + j] = _f32(inputs["em_gamma"][j])
        cols24[:, _CC["embe"] + j] = _f32(inputs["em_beta"][j])
    cols24[:, _CC["f1b"]:_CC["f1b"] + 2] = \
        _f32(inputs["fcf_b1"]).reshape(2, 128).T
    cols24[:, _CC["f2b"]:_CC["f2b"] + 2] = \
        _f32(inputs["fcf_b2"]).reshape(2, 128).T

    W2 = _f32(inputs["fcf_W2"])
    L_f2 = np.zeros((2, 2, 128, 128), np.float32)
    for h in range(2):
        for k in range(2):
            L_f2[h, k] = W2.T[k * 128:(k + 1) * 128, h * 128:(h + 1) * 128]

    # fp8 half-identity pair for DoubleRow injection (scaled by 1/FP8_SCALE)
    eye = np.eye(128, dtype=np.float32) * (0.5 / FP8_SCALE)
    i128h8 = np.concatenate([eye, eye], axis=1).astype(
        ml_dtypes.float8_e4m3)

    # stage-1 phase constants: s_t = 4 * FREQS_t / (2*pi), 3-level bf16
    # split of SD/xk so the phase matmul runs in bf16 at fp32-like accuracy
    s_vals = (4.0 * FREQS / (2 * np.pi))         # [5] float64
    SD = np.zeros((3, 47), np.float64)
    for j in range(3):
        for t in range(5):
            SD[j, 5 * j + t] = s_vals[t]
            SD[j, 32 + 5 * j + t] = s_vals[t]
    Sa, Sb, Sc = _split3(SD)
    SD18 = np.concatenate([Sa, Sa, Sb, Sa, Sc, Sb], axis=0).astype(BF)

    com = {
        "feats_row": _bf(feats.reshape(1, 768)),
        "SD18": np.ascontiguousarray(SD18),
        "i128h8": np.ascontiguousarray(i128h8),
        "i128": _bf(np.eye(128, dtype=np.float32)),
        "L_G1P3": _bf(L_G1P3), "L_Ps3": _bf(L_Ps3),
        "L_Wg1": _bf(L_Wg1), "L_Wg2": _bf(L_Wg2),
        "L_nWk": _bf(L_nWk), "L_Wv": _bf(L_Wv), "L_G1Q": _bf(L_G1Q),
        "L_pd9": _bf(L_pd9), "rows2": _bf(rows2), "cols24": cols24,
        "L_em1": _bf(np.stack([_f32(inputs["em_W1"][j]).T for j in range(NF)])),
        "L_em2": _bf(np.stack([_f32(inputs["em_W2"][j]).T for j in range(NF)])),
        "L_f1": _bf(_f32(inputs["fcf_W1"]).T.reshape(128, 2, 128)
                    .transpose(1, 0, 2)),
        "L_f2": _bf(L_f2),
    }

    in_maps = []
    for c in range(8):
        b, qo = c // 4, (c % 4) * 96
        xk = xyz[b].T                      # [3, 384]
        # per-chunk lhsT [47, 96]: rows 0-14 pair with sin(2*pi*pf) and
        # carry cos(2*pi*of); rows 32-46 pair with cos(2*pi*pf) and carry
        # sin(2*pi*of); rows 15-31 are zero padding.
        L_OF2 = np.zeros((32, 47, 96), np.float64)
        for cch in range(32):
            for qr in range(3):
                qg = qo + 3 * cch + qr
                for j in range(3):
                    for t in range(10):
                        col = 32 * qr + 10 * j + t
                        ph = 5 * j + (t % 5)
                        off = (0.25 if t >= 5 else 0.0) \
                            - s_vals[t % 5] * np.float64(xyz[b, qg, j])
                        ang = 2 * np.pi * (off - np.round(off))
                        L_OF2[cch, ph, col] = np.cos(ang)
                        L_OF2[cch, 32 + ph, col] = np.sin(ang)
        m = dict(com)
        Xa, Xb, Xc = _split3(xk)
        m["xk18"] = np.ascontiguousarray(
            np.concatenate([Xa, Xb, Xa, Xc, Xa, Xb], axis=0).astype(BF))
        m["L_OF2"] = _bf(L_OF2.transpose(1, 0, 2).reshape(47, 32 * 96))
        m["xk_b"] = _bf(xk)
        m["xq_b"] = _bf(xk[:, qo:qo + 96])
        m["feats_q"] = _bf(feats[b, qo:qo + 96].reshape(1, 96))
        in_maps.append(m)
    return in_maps


def kernel(**inputs):
    from concourse.bass_utils import run_bass_kernel_spmd

    nc = _build()
    in_maps = _prep_inputs(inputs)
    res = run_bass_kernel_spmd(nc, in_maps, list(range(8)))
    return np.asarray(res.results[0]["out"], np.float32)


if __name__ == "__main__":
    print("smoke build only")
